# revision 7
# baseline (speedup 1.0000x reference)
"""Censored-loss kernel for Trainium2, data-parallel over 8 NeuronCores.

Math (per reference):
    per_t = targets.sum(-1)                      # [B, T]
    mask  = prefix mask: mask[t] = 1 iff any per_t[t'] > 0 for t' >= t
    censor_p = 1 - outputs.sum(-1)
    loss  = sum(mask * (targets[:,:,0]*ln(censor_p+eps)
                        + sum_v targets[:,:,1+v]*ln(outputs[:,:,v]+eps)))
    count = sum(mask)
    result = -loss / max(count, 1)   (0 if count == 0)

Key simplifications (targets >= 0 by construction):
  * Positions with mask==0 have targets==0 exactly, so they contribute 0 to
    the loss numerator -> no mask needed for the loss sum.
  * count = #positions whose targets are nonzero; we count t0 > 0.

Inputs staged to fp16 on host (halves HBM traffic); targets reordered to
[t0-block | t1..t4 blocks] per row so all on-chip accesses are contiguous.

Engine split per 128-row tile (16 tiles per core), balanced so every
engine sits at or below the ~2.95us/tile DMA floor:
  DVE   (~2.5us): s2 = pairwise censor add (fp16 TT, 2x packed),
                  prod = targets*logt (fp16 TT, 2x),
                  count = tensor_scalar is_gt (4x) with f32 accum_out
  GpSimd(~1.7us): ms = -(s2a+s2b) via scalar_tensor_tensor,
                  fold01 = prod_c0 + prod_c1 via scalar_tensor_tensor
  ACT   (~2.7us): logt[T:] = Ln(o+eps), logt[:T] = Ln(ms+1.0)
  PE    (~2.9us): 4 accumulating ones-matmuls (fold01, c2, c3, c4) into
                  2 alternating [1, 512] f32 PSUM banks
Host: f64 reduction of the [1,1024] loss partials and [P,16] count
partials, then -loss/max(count,1).
"""

import sys

if "/opt/trn_rl_repo" not in sys.path:
    sys.path.insert(0, "/opt/trn_rl_repo")

import numpy as np

import concourse.bacc as bacc
import concourse.mybir as mybir
import concourse.tile as tile
from concourse.bass_utils import run_bass_kernel_spmd

N_CORES = 8
B, T, V = 16384, 512, 5
ROWS = B // N_CORES           # rows per core
P = 128                       # SBUF partitions
NTILES = ROWS // P            # tiles per core
OW = T * (V - 1)              # outputs row width (flattened)
TW = T * V                    # targets row width (flattened)
EPS = 1e-8
F32 = mybir.dt.float32
F16 = mybir.dt.float16
BF16 = mybir.dt.bfloat16
NPF16 = np.float16
ACT = mybir.ActivationFunctionType
ALU = mybir.AluOpType


def build_nc(rows=ROWS):
    ntiles = rows // P
    nc = bacc.Bacc("TRN2", debug=False, num_devices=N_CORES)
    o_d = nc.dram_tensor("outputs", [rows, OW], F16, kind="ExternalInput")
    t_d = nc.dram_tensor("targets", [rows, TW], F16, kind="ExternalInput")
    loss_d = nc.dram_tensor("loss_acc", [1, 2 * T], F32, kind="ExternalOutput")
    cnt_d = nc.dram_tensor("cnt_acc", [P, ntiles], F32, kind="ExternalOutput")

    o_tiled = o_d.ap().rearrange("(n p) m -> n p m", p=P)
    t_tiled = t_d.ap().rearrange("(n p) m -> n p m", p=P)

    with tile.TileContext(nc) as tc:
        with (
            tc.tile_pool(name="inp", bufs=10) as inp,
            tc.tile_pool(name="mid", bufs=4) as mid,
            tc.tile_pool(name="big", bufs=3) as big,
            tc.tile_pool(name="sml", bufs=3) as sml,
            tc.tile_pool(name="acc", bufs=1) as accp,
            tc.tile_pool(name="ps", bufs=1, space="PSUM") as psp,
        ):
            cnt_acc = accp.tile([P, ntiles], F32)
            eps_b = accp.tile([P, 1], F32)
            nc.vector.memset(eps_b[:], EPS)
            ones = accp.tile([P, 1], BF16)
            nc.vector.memset(ones[:], 1.0)
            # two alternating loss accumulators (separate PSUM banks so
            # consecutive accumulating matmuls can pipeline)
            loss_ps0 = psp.tile([1, T], F32, tag="lps0")
            loss_ps1 = psp.tile([1, T], F32, tag="lps1")
            loss_ps = [loss_ps0, loss_ps1]
            nmm = 0  # loss matmul counter across the whole kernel
            n_loss_mm = 4 * ntiles

            o_t, tg_t, ms_t = {}, {}, {}

            def load_stage(i):
                """DMA tile i, then censor pair-add on DVE and the negated
                final censor sum on GpSimd -- all ahead of the consuming
                ACT/loss ops."""
                o = inp.tile([P, OW], F16, tag="o")
                nc.sync.dma_start(o[:], o_tiled[i])
                tg = inp.tile([P, TW], F16, tag="tg")
                nc.sync.dma_start(tg[:], t_tiled[i])
                o_t[i], tg_t[i] = o, tg
                # s2[p, t, 0:2] = (o0+o2, o1+o3) -- consecutive-pair adds in
                # fp16 hit the DVE 2x packed mode
                s2 = mid.tile([P, T * 2], F16, tag="s2")
                s2v = s2[:].rearrange("p (t v) -> p t v", v=2)
                o3 = o[:].rearrange("p (t v) -> p t v", v=V - 1)
                nc.vector.tensor_tensor(
                    s2v, o3[:, :, 0:2], o3[:, :, 2:4], op=ALU.add
                )
                # s = s2a + s2b (full censor sum), on the otherwise-idle
                # GpSimd engine
                ms = mid.tile([P, T], F16, tag="ms")
                nc.gpsimd.tensor_tensor(
                    ms[:], s2v[:, :, 0], s2v[:, :, 1], op=ALU.add
                )
                ms_t[i] = ms

            load_stage(0)
            load_stage(1)
            for i in range(ntiles):
                if i + 2 < ntiles:
                    load_stage(i + 2)

                o, tg, ms = o_t.pop(i), tg_t.pop(i), ms_t.pop(i)

                # log tile, same [t0|tv] layout as the reordered targets.
                # Ln(o+eps) first: it only depends on the DMA, so ACT never
                # stalls on the DVE/GpSimd censor chain.
                logt = big.tile([P, TW], F16, tag="logt")
                nc.scalar.activation(
                    logt[:][:, T:TW], o[:], ACT.Ln, bias=eps_b[:]
                )
                # f32(1 + 1e-8) == 1.0 exactly, so bias=1.0 == 1+eps
                nc.scalar.activation(
                    logt[:][:, 0:T], ms[:], ACT.Ln, bias=1.0, scale=-1.0
                )

                # count: is_gt at DVE 4x with free f32 accumulation
                sgn = sml.tile([P, T], BF16, tag="sgn")
                nc.vector.tensor_scalar(
                    out=sgn[:], in0=tg[:][:, 0:T],
                    scalar1=0.0, scalar2=0.0, op0=ALU.is_gt, op1=ALU.add,
                    accum_out=cnt_acc[:, i : i + 1],
                )

                # loss product (DVE fp16 2x): prod = targets * logt
                prod = big.tile([P, TW], BF16, tag="prod")
                nc.vector.tensor_tensor(prod[:], tg[:], logt[:], op=ALU.mult)

                # fold chunks 0+1 on DVE (bf16 contiguous TT, 2x) so PE does
                # 4 loss matmuls instead of 5
                fold = sml.tile([P, T], BF16, tag="fold")
                nc.vector.tensor_tensor(
                    fold[:], prod[:][:, 0:T], prod[:][:, T : 2 * T],
                    op=ALU.add,
                )

                # PE: accumulate partition+chunk sums into PSUM [1, T] accs
                rhss = [fold[:]] + [
                    prod[:][:, c * T : (c + 1) * T] for c in range(2, V)
                ]
                for rhs in rhss:
                    nc.tensor.matmul(
                        loss_ps[nmm % 2][:],
                        ones[:],
                        rhs,
                        start=(nmm < 2),
                        stop=(nmm >= n_loss_mm - 2),
                    )
                    nmm += 1

            loss_sb = accp.tile([1, 2 * T], F32)
            nc.scalar.copy(loss_sb[:, 0:T], loss_ps[0][:])
            nc.scalar.copy(loss_sb[:, T : 2 * T], loss_ps[1][:])
            nc.sync.dma_start(loss_d.ap(), loss_sb[:])
            nc.sync.dma_start(cnt_d.ap(), cnt_acc[:])
    nc.compile()
    return nc


_NC_CACHE = {}


def _get_nc(rows=ROWS):
    if rows not in _NC_CACHE:
        _NC_CACHE[rows] = build_nc(rows)
    return _NC_CACHE[rows]


def pack_inputs(outputs, targets):
    """fp16 staging + per-row [t0-block | tv-block] reorder of targets."""
    o = np.asarray(outputs).reshape(N_CORES, ROWS, OW).astype(NPF16)
    t3 = np.asarray(targets).reshape(N_CORES, ROWS, T, V).astype(NPF16)
    tg = np.concatenate(
        [t3[:, :, :, 0], t3[:, :, :, 1:].reshape(N_CORES, ROWS, OW)], axis=2
    )
    return o, tg


def run_spmd(outputs, targets, trace=False, **kwargs):
    o, tg = pack_inputs(outputs, targets)
    in_maps = [{"outputs": o[k], "targets": tg[k]} for k in range(N_CORES)]
    nc = _get_nc()
    res = run_bass_kernel_spmd(
        nc, in_maps, core_ids=list(range(N_CORES)), trace=trace, **kwargs
    )
    loss = sum(r["loss_acc"].astype(np.float64).sum() for r in res.results)
    cnt = sum(r["cnt_acc"].astype(np.float64).sum() for r in res.results)
    return loss, cnt, res


def kernel(outputs, targets):
    loss, cnt, _ = run_spmd(outputs, targets)
    if cnt > 0:
        return np.float32(-loss / max(cnt, 1.0))
    return np.float32(0.0)
